# revision 29
# baseline (speedup 1.0000x reference)
"""Trainium2 Bass kernel for nn_AddPoolingFusion.

Reference computation (b=16, l1=l2=2048, d1=d2=d3=768):
    y1  = x1 @ W1.T + b1                      # [b, l1, d3]
    y2  = x2 @ W2.T + b2                      # [b, l2, d3]
    out = y1 + mean(y2, axis=1, keepdims=True)

Because the mean over l2 commutes with the linear layer:
    out[b,i,:] = x1[b,i] @ W1.T + c[b]
    c[b]       = (b1 + b2) + mean_j(x2[b,j]) @ W2.T

Strategy: data-parallel over batch, 2 batches per core, no collectives.
Per core the only heavy compute is the x1 matmul (bf16 on TensorE) and
the x2 mean (DVE accumulate + tiny ones-matmul partition reduce); the
kernel is HBM-bandwidth / TensorE bound.

Host-side prep (layout/dtype only): all tensors are pre-arranged into
partition-major SBUF images so every DMA descriptor covers a 6-48 KB
contiguous run (small runs make the SDMA engines descriptor-bound at
~240 GB/s; big runs reach HBM line rate). x1 is pre-transposed so the
contraction dim lands on SBUF partitions, and x1/x2/weights are pre-cast
to bf16 (the TensorE matmul runs in bf16 either way; rel err stays ~3e-3
vs the 2e-2 gate). The output is stored as bf16 and upcast on the host.
"""

import os
import sys

import numpy as np

# concourse normally comes from the axon site overlay already on sys.path;
# append /opt/trn_rl_repo as a fallback only.
if "/opt/trn_rl_repo" not in sys.path:
    sys.path.append("/opt/trn_rl_repo")

N_CORES = 8
B_PER_CORE = 2
L = 2048
D = 768  # d1 == d2 == d3 == 768
P = 128
NCH = D // P  # 6 contraction chunks
M = B_PER_CORE * L  # 4096 rows per core
TPG = 8  # m-tiles per x1 DMA group
NGRP = (M // P) // TPG  # 4 groups
SPT = 4  # m-tiles per output store
NST = (M // P) // SPT  # 8 stores


def build_nc(debug=False, explicit_ldw=False):
    import concourse.bacc as bacc
    import concourse.mybir as mybir
    import concourse.tile as tile

    f32 = mybir.dt.float32
    bf16 = mybir.dt.bfloat16
    add = mybir.AluOpType.add

    nc = bacc.Bacc(None, target_bir_lowering=False, debug=debug)

    # All layouts are partition-major SBUF images (see make_in_maps).
    x1h = nc.declare_dram_parameter("x1h", [2 * NGRP, P, NCH, TPG * P // 2], bf16, isOutput=False)
    x2h = nc.declare_dram_parameter("x2h", [B_PER_CORE, P, L // P, D], bf16, isOutput=False)
    w1h = nc.declare_dram_parameter("w1h", [P, NCH, D], bf16, isOutput=False)
    w2h = nc.declare_dram_parameter("w2h", [P, NCH, D], bf16, isOutput=False)
    bsum = nc.declare_dram_parameter("bsum", [1, D], f32, isOutput=False)
    outh = nc.declare_dram_parameter("outh", [NST, P, SPT, D], bf16, isOutput=True)

    with tile.TileContext(nc) as tc:
        with (
            tc.tile_pool(name="const", bufs=1) as const,
            tc.tile_pool(name="x2p", bufs=4) as x2p,
            tc.tile_pool(name="x2big", bufs=1) as x2big,
            tc.tile_pool(name="x1p", bufs=4) as x1p,
            tc.tile_pool(name="yp", bufs=2) as yp,
            tc.tile_pool(name="small", bufs=1) as small,
            tc.tile_pool(name="psY", bufs=3, space="PSUM") as psY,
            tc.tile_pool(name="psM", bufs=1, space="PSUM") as psM,
        ):
            # ---- weights / constants ----
            # DMA ring split: x1 + W1 on the Sync HWDGE ring; x2 + W2 +
            # stores on the Scalar HWDGE ring.
            w1sb = const.tile([P, NCH, D], bf16)
            nc.scalar.dma_start(w1sb[:], w1h[:])
            ones_sb = const.tile([P, 1], bf16)
            nc.vector.memset(ones_sb[:], 1.0)

            # ---- x2 loads + per-batch accumulate (DVE) ----
            # Ring FIFO order on the Scalar ring encodes the schedule:
            # x2 batch 0 (c[0]-critical) -> w2/bsum -> x2 batch 1 -> stores.
            accs = []

            def x2_acc(b, nchunk, pool):
                with nc.named_scope(f"x2_acc{b}"):
                    acc = small.tile([P, D], bf16, tag=f"acc{b}")
                    tpc = (L // P) // nchunk
                    first = True
                    for h in range(nchunk):
                        st = pool.tile([P, tpc, D], bf16, tag=f"x2st{b}")
                        dma = nc.scalar.dma_start(
                            st[:], x2h[b, :, tpc * h : tpc * (h + 1), :]
                        )
                        for t in range(tpc):
                            if first:
                                nc.vector.tensor_copy(acc[:], st[:, 0, :])
                                first = False
                            else:
                                nc.vector.tensor_tensor(
                                    acc[:], acc[:], st[:, t, :], op=add
                                )
                    accs.append(acc)

            x2_acc(0, 4, x2p)
            w2sb = const.tile([P, NCH, D], bf16)
            nc.scalar.dma_start(w2sb[:], w2h[:])
            bsum_sb = const.tile([1, D], f32)
            nc.scalar.dma_start(bsum_sb[:], bsum[:])
            x2_acc(1, 1, x2big)

            def c_path(b):
                # partition-major sum: xbt[p, c] = (1/L) sum_j x2[b, j, c*128+p]
                with nc.named_scope(f"c_path{b}"):
                    acc = accs[b]
                    xbt = small.tile([P, NCH], bf16, tag=f"xbt{b}")
                    for c in range(NCH):
                        px = psM.tile([P, 1], f32, tag="pscratch")
                        nc.tensor.matmul(
                            px[:], acc[:, c * P : (c + 1) * P], ones_sb[:],
                            start=True, stop=True,
                        )
                        # fold the 1/L mean scale in via the copy
                        nc.vector.tensor_scalar_mul(xbt[:, c : c + 1], px[:], 1.0 / L)
                    # c_lin = xbar2 @ W2.T  (tiny matmul, K=768, M=1, N=768)
                    pc = psM.tile([1, D], f32, tag="pscratch")
                    for c in range(NCH):
                        nc.tensor.matmul(
                            pc[:, 0:512], xbt[:, c : c + 1], w2sb[:, c, 0:512],
                            start=(c == 0), stop=(c == NCH - 1),
                        )
                    for c in range(NCH):
                        nc.tensor.matmul(
                            pc[:, 512:768], xbt[:, c : c + 1], w2sb[:, c, 512:768],
                            start=(c == 0), stop=(c == NCH - 1),
                        )
                    cs = small.tile([1, D], bf16, tag=f"cs{b}")
                    nc.vector.tensor_tensor(cs[:], pc[:], bsum_sb[:], op=add)
                    cr = small.tile([P, D], bf16, tag=f"cr{b}")
                    nc.gpsimd.partition_broadcast(cr[:], cs[:])
                    return cr

            # ---- main matmul: out = x1 @ W1.T + c[b] ----
            c_rep = [None, None]

            HG = TPG * P // 2  # columns per half-group load

            def group(g, defer_bias=False, prev_dma=None):
                with nc.named_scope(f"grp{g}"):
                    xs = x1p.tile([P, NCH, TPG * P], bf16, tag="xs")
                    dma = prev_dma
                    for half in range(2):
                        d = nc.sync.dma_start(
                            xs[:, :, half * HG : (half + 1) * HG], x1h[2 * g + half]
                        )
                        if dma is not None:
                            # just-in-time pacing: each x1 half-load starts
                            # only after the previous one fully lands, so
                            # queued loads never steal HBM bandwidth from the
                            # transfer the TensorE is actually waiting on
                            tile.add_dep_helper(
                                d.ins, dma.ins, sync=True,
                                reason="pace x1 loads just-in-time",
                            )
                        dma = d
                    ys = yp.tile([P, TPG, D], bf16)
                    for t in range(TPG):
                        mt = g * TPG + t
                        b = mt // (L // P)
                        py_ = psY.tile([P, D], f32)
                        xw = xs[:, :, t * P : (t + 1) * P]
                        for c in range(NCH):
                            if explicit_ldw:
                                nc.tensor.ldweights(xw[:, c, :])
                            nc.tensor.matmul(
                                py_[:, 0:512], xw[:, c, :], w1sb[:, c, 0:512],
                                start=(c == 0), stop=(c == NCH - 1),
                            )
                            nc.tensor.matmul(
                                py_[:, 512:768], xw[:, c, :], w1sb[:, c, 512:768],
                                start=(c == 0), stop=(c == NCH - 1),
                            )
                        if defer_bias:
                            # plain evac: never lets PSUM recycling wait on c
                            nc.vector.tensor_copy(ys[:, t, :], py_[:])
                        else:
                            # fused evac + bias add
                            nc.vector.tensor_tensor(
                                ys[:, t, :], py_[:], c_rep[b][:], op=add
                            )
                            if t % SPT == SPT - 1:
                                nc.sync.dma_start(
                                    outh[mt // SPT], ys[:, t - SPT + 1 : t + 1, :]
                                )
                    group.last_dma = dma
                    if defer_bias:
                        b = (g * TPG) // (L // P)
                        for t in range(TPG):
                            mt = g * TPG + t
                            nc.vector.tensor_tensor(
                                ys[:, t, :], ys[:, t, :], c_rep[b][:], op=add
                            )
                            if t % SPT == SPT - 1:
                                nc.sync.dma_start(
                                    outh[mt // SPT], ys[:, t - SPT + 1 : t + 1, :]
                                )

            # batch-0 c-path first (gates groups 0-1), batch-1 c-path after
            # group 1 so its PE slot lands when its inputs are long ready.
            # Group 0 runs while c[0] is still in flight -> deferred bias.
            c_rep[0] = c_path(0)
            group(0, defer_bias=True)
            group(1, prev_dma=group.last_dma)
            c_rep[1] = c_path(1)
            group(2, defer_bias=True, prev_dma=group.last_dma)
            group(3, prev_dma=group.last_dma)

    return nc


def make_in_maps(x1, x2, W1, b1, W2, b2):
    import ml_dtypes

    bf16 = ml_dtypes.bfloat16

    def wlayout(W):
        # [e, d] -> W.T [d, e] -> [p, c, e] with d = c*128 + p
        wt = np.ascontiguousarray(W.T).reshape(NCH, P, D).transpose(1, 0, 2)
        return np.ascontiguousarray(wt).astype(bf16)

    w1h = wlayout(W1)
    w2h = wlayout(W2)
    bsum_h = np.ascontiguousarray((b1 + b2).reshape(1, D).astype(np.float32))
    in_maps = []
    for k in range(N_CORES):
        x1_s = x1[k * B_PER_CORE : (k + 1) * B_PER_CORE]  # [2, 2048, 768]
        x2_s = x2[k * B_PER_CORE : (k + 1) * B_PER_CORE]
        # x1t [d, m] with col m = b*2048 + i, then group-major partition image
        x1t = np.transpose(x1_s, (2, 0, 1)).reshape(D, M)
        x1h = np.ascontiguousarray(
            x1t.reshape(NCH, P, 2 * NGRP, TPG * P // 2).transpose(2, 1, 0, 3)
        ).astype(bf16)  # [half_group, p, c, m_in_half]
        # x2 [b, j, d] with j = t*128 + p -> [b, p, t, d]
        x2h = np.ascontiguousarray(
            x2_s.reshape(B_PER_CORE, L // P, P, D).transpose(0, 2, 1, 3)
        ).astype(bf16)
        in_maps.append(
            {"x1h": x1h, "x2h": x2h, "w1h": w1h, "w2h": w2h, "bsum": bsum_h}
        )
    return in_maps


def kernel(x1, x2, W1, b1, W2, b2, trace=False, explicit_ldw=False):
    from concourse.bass_utils import run_bass_kernel_spmd

    nc = build_nc(debug=False, explicit_ldw=explicit_ldw)
    nc.finalize()
    in_maps = make_in_maps(x1, x2, W1, b1, W2, b2)
    res = run_bass_kernel_spmd(
        nc, in_maps, core_ids=list(range(N_CORES)), trace=trace
    )
    shards = []
    for k in range(N_CORES):
        oh = res.results[k]["outh"]  # [NST, P, SPT, D] bf16, row = (s*SPT+t)*128+p
        flat = (
            oh.astype(np.float32).transpose(0, 2, 1, 3).reshape(M, D)
        )
        shards.append(flat.reshape(B_PER_CORE, L, D))
    out = np.concatenate(shards, axis=0)
    if trace:
        kernel.last_result = res
    return out


# revision 30
# speedup vs baseline: 1.1411x; 1.1411x over previous
"""Trainium2 Bass kernel for nn_AddPoolingFusion.

Reference computation (b=16, l1=l2=2048, d1=d2=d3=768):
    y1  = x1 @ W1.T + b1                      # [b, l1, d3]
    y2  = x2 @ W2.T + b2                      # [b, l2, d3]
    out = y1 + mean(y2, axis=1, keepdims=True)

Because the mean over l2 commutes with the linear layer:
    out[b,i,:] = x1[b,i] @ W1.T + c[b]
    c[b]       = (b1 + b2) + mean_j(x2[b,j]) @ W2.T

Strategy: data-parallel over batch, 2 batches per core, no collectives.
Per core the only heavy compute is the x1 matmul (bf16 on TensorE) and
the x2 mean (DVE accumulate + tiny ones-matmul partition reduce); the
kernel is HBM-bandwidth / TensorE bound.

Host-side prep (layout/dtype only): all tensors are pre-arranged into
partition-major SBUF images so every DMA descriptor covers a 6-48 KB
contiguous run (small runs make the SDMA engines descriptor-bound at
~240 GB/s; big runs reach HBM line rate). x1 is pre-transposed so the
contraction dim lands on SBUF partitions, and x1/x2/weights are pre-cast
to bf16 (the TensorE matmul runs in bf16 either way; rel err stays ~3e-3
vs the 2e-2 gate). The output is stored as bf16 and upcast on the host.
"""

import os
import sys

import numpy as np

# concourse normally comes from the axon site overlay already on sys.path;
# append /opt/trn_rl_repo as a fallback only.
if "/opt/trn_rl_repo" not in sys.path:
    sys.path.append("/opt/trn_rl_repo")

N_CORES = 8
B_PER_CORE = 2
L = 2048
D = 768  # d1 == d2 == d3 == 768
P = 128
NCH = D // P  # 6 contraction chunks
M = B_PER_CORE * L  # 4096 rows per core
TPG = 8  # m-tiles per x1 DMA group
NGRP = (M // P) // TPG  # 4 groups
SPT = 4  # m-tiles per output store
NST = (M // P) // SPT  # 8 stores


def build_nc(debug=False, explicit_ldw=False):
    import concourse.bacc as bacc
    import concourse.mybir as mybir
    import concourse.tile as tile

    f32 = mybir.dt.float32
    bf16 = mybir.dt.bfloat16
    add = mybir.AluOpType.add

    nc = bacc.Bacc(None, target_bir_lowering=False, debug=debug)

    # All layouts are partition-major SBUF images (see make_in_maps).
    x1h = nc.declare_dram_parameter("x1h", [NGRP, P, NCH, TPG * P], bf16, isOutput=False)
    x2h = nc.declare_dram_parameter("x2h", [B_PER_CORE, P, L // P, D], bf16, isOutput=False)
    w1h = nc.declare_dram_parameter("w1h", [P, NCH, D], bf16, isOutput=False)
    w2h = nc.declare_dram_parameter("w2h", [P, NCH, D], bf16, isOutput=False)
    bsum = nc.declare_dram_parameter("bsum", [1, D], f32, isOutput=False)
    outh = nc.declare_dram_parameter("outh", [NST, P, SPT, D], bf16, isOutput=True)

    with tile.TileContext(nc) as tc:
        with (
            tc.tile_pool(name="const", bufs=1) as const,
            tc.tile_pool(name="x2p", bufs=4) as x2p,
            tc.tile_pool(name="x2big", bufs=1) as x2big,
            tc.tile_pool(name="x1p", bufs=4) as x1p,
            tc.tile_pool(name="yp", bufs=2) as yp,
            tc.tile_pool(name="small", bufs=1) as small,
            tc.tile_pool(name="psY", bufs=3, space="PSUM") as psY,
            tc.tile_pool(name="psM", bufs=1, space="PSUM") as psM,
        ):
            # ---- weights / constants ----
            # DMA ring split: x1 + W1 on the Sync HWDGE ring; x2 + W2 +
            # stores on the Scalar HWDGE ring.
            w1sb = const.tile([P, NCH, D], bf16)
            nc.scalar.dma_start(w1sb[:], w1h[:])
            ones_sb = const.tile([P, 1], bf16)
            nc.vector.memset(ones_sb[:], 1.0)

            # ---- x2 loads + per-batch accumulate (DVE) ----
            # Ring FIFO order on the Scalar ring encodes the schedule:
            # x2 batch 0 (c[0]-critical) -> w2/bsum -> x2 batch 1 -> stores.
            accs = []

            def x2_acc(b, nchunk, pool):
                with nc.named_scope(f"x2_acc{b}"):
                    acc = small.tile([P, D], bf16, tag=f"acc{b}")
                    tpc = (L // P) // nchunk
                    first = True
                    for h in range(nchunk):
                        st = pool.tile([P, tpc, D], bf16, tag=f"x2st{b}")
                        dma = nc.scalar.dma_start(
                            st[:], x2h[b, :, tpc * h : tpc * (h + 1), :]
                        )
                        for t in range(tpc):
                            if first:
                                nc.vector.tensor_copy(acc[:], st[:, 0, :])
                                first = False
                            else:
                                nc.vector.tensor_tensor(
                                    acc[:], acc[:], st[:, t, :], op=add
                                )
                    accs.append(acc)

            x2_acc(0, 4, x2p)
            w2sb = const.tile([P, NCH, D], bf16)
            nc.scalar.dma_start(w2sb[:], w2h[:])
            bsum_sb = const.tile([1, D], f32)
            nc.scalar.dma_start(bsum_sb[:], bsum[:])
            x2_acc(1, 1, x2big)

            def c_path(b):
                # partition-major sum: xbt[p, c] = (1/L) sum_j x2[b, j, c*128+p]
                with nc.named_scope(f"c_path{b}"):
                    acc = accs[b]
                    xbt = small.tile([P, NCH], bf16, tag=f"xbt{b}")
                    for c in range(NCH):
                        px = psM.tile([P, 1], f32, tag="pscratch")
                        nc.tensor.matmul(
                            px[:], acc[:, c * P : (c + 1) * P], ones_sb[:],
                            start=True, stop=True,
                        )
                        # fold the 1/L mean scale in via the copy
                        nc.vector.tensor_scalar_mul(xbt[:, c : c + 1], px[:], 1.0 / L)
                    # c_lin = xbar2 @ W2.T  (tiny matmul, K=768, M=1, N=768)
                    pc = psM.tile([1, D], f32, tag="pscratch")
                    for c in range(NCH):
                        nc.tensor.matmul(
                            pc[:, 0:512], xbt[:, c : c + 1], w2sb[:, c, 0:512],
                            start=(c == 0), stop=(c == NCH - 1),
                        )
                    for c in range(NCH):
                        nc.tensor.matmul(
                            pc[:, 512:768], xbt[:, c : c + 1], w2sb[:, c, 512:768],
                            start=(c == 0), stop=(c == NCH - 1),
                        )
                    cs = small.tile([1, D], bf16, tag=f"cs{b}")
                    nc.vector.tensor_tensor(cs[:], pc[:], bsum_sb[:], op=add)
                    cr = small.tile([P, D], bf16, tag=f"cr{b}")
                    nc.gpsimd.partition_broadcast(cr[:], cs[:])
                    return cr

            # ---- main matmul: out = x1 @ W1.T + c[b] ----
            c_rep = [None, None]

            def group(g, defer_bias=False, prev_dma=None):
                with nc.named_scope(f"grp{g}"):
                    xs = x1p.tile([P, NCH, TPG * P], bf16, tag="xs")
                    dma = nc.sync.dma_start(xs[:], x1h[g])
                    if prev_dma is not None:
                        # just-in-time pacing: an x1 load starts only after an
                        # earlier one fully lands, so queued loads never steal
                        # HBM bandwidth from the transfer TensorE waits on
                        tile.add_dep_helper(
                            dma.ins, prev_dma.ins, sync=True,
                            reason="pace x1 loads just-in-time",
                        )
                    ys = yp.tile([P, TPG, D], bf16)
                    for t in range(TPG):
                        mt = g * TPG + t
                        b = mt // (L // P)
                        py_ = psY.tile([P, D], f32)
                        xw = xs[:, :, t * P : (t + 1) * P]
                        for c in range(NCH):
                            if explicit_ldw:
                                nc.tensor.ldweights(xw[:, c, :])
                            nc.tensor.matmul(
                                py_[:, 0:512], xw[:, c, :], w1sb[:, c, 0:512],
                                start=(c == 0), stop=(c == NCH - 1),
                            )
                            nc.tensor.matmul(
                                py_[:, 512:768], xw[:, c, :], w1sb[:, c, 512:768],
                                start=(c == 0), stop=(c == NCH - 1),
                            )
                        if defer_bias:
                            # plain evac: never lets PSUM recycling wait on c
                            nc.vector.tensor_copy(ys[:, t, :], py_[:])
                        else:
                            # fused evac + bias add
                            nc.vector.tensor_tensor(
                                ys[:, t, :], py_[:], c_rep[b][:], op=add
                            )
                            if t % SPT == SPT - 1:
                                nc.sync.dma_start(
                                    outh[mt // SPT], ys[:, t - SPT + 1 : t + 1, :]
                                )
                    group.last_dma = dma
                    if defer_bias:
                        b = (g * TPG) // (L // P)
                        for t in range(TPG):
                            mt = g * TPG + t
                            nc.vector.tensor_tensor(
                                ys[:, t, :], ys[:, t, :], c_rep[b][:], op=add
                            )
                            if t % SPT == SPT - 1:
                                nc.sync.dma_start(
                                    outh[mt // SPT], ys[:, t - SPT + 1 : t + 1, :]
                                )

            # batch-0 c-path first (gates groups 0-1), batch-1 c-path after
            # group 1 so its PE slot lands when its inputs are long ready.
            # Group 0 runs while c[0] is still in flight -> deferred bias.
            c_rep[0] = c_path(0)
            group(0, defer_bias=True)
            group(1, prev_dma=group.last_dma)
            dma_g1 = group.last_dma
            c_rep[1] = c_path(1)
            group(2, defer_bias=True, prev_dma=dma_g1)
            group(3, prev_dma=dma_g1)

    return nc


def make_in_maps(x1, x2, W1, b1, W2, b2):
    import ml_dtypes

    bf16 = ml_dtypes.bfloat16

    def wlayout(W):
        # [e, d] -> W.T [d, e] -> [p, c, e] with d = c*128 + p
        wt = np.ascontiguousarray(W.T).reshape(NCH, P, D).transpose(1, 0, 2)
        return np.ascontiguousarray(wt).astype(bf16)

    w1h = wlayout(W1)
    w2h = wlayout(W2)
    bsum_h = np.ascontiguousarray((b1 + b2).reshape(1, D).astype(np.float32))
    in_maps = []
    for k in range(N_CORES):
        x1_s = x1[k * B_PER_CORE : (k + 1) * B_PER_CORE]  # [2, 2048, 768]
        x2_s = x2[k * B_PER_CORE : (k + 1) * B_PER_CORE]
        # x1t [d, m] with col m = b*2048 + i, then group-major partition image
        x1t = np.transpose(x1_s, (2, 0, 1)).reshape(D, M)
        x1h = np.ascontiguousarray(
            x1t.reshape(NCH, P, NGRP, TPG * P).transpose(2, 1, 0, 3)
        ).astype(bf16)  # [g, p, c, m_in_group]
        # x2 [b, j, d] with j = t*128 + p -> [b, p, t, d]
        x2h = np.ascontiguousarray(
            x2_s.reshape(B_PER_CORE, L // P, P, D).transpose(0, 2, 1, 3)
        ).astype(bf16)
        in_maps.append(
            {"x1h": x1h, "x2h": x2h, "w1h": w1h, "w2h": w2h, "bsum": bsum_h}
        )
    return in_maps


def kernel(x1, x2, W1, b1, W2, b2, trace=False, explicit_ldw=False):
    from concourse.bass_utils import run_bass_kernel_spmd

    nc = build_nc(debug=False, explicit_ldw=explicit_ldw)
    nc.finalize()
    in_maps = make_in_maps(x1, x2, W1, b1, W2, b2)
    res = run_bass_kernel_spmd(
        nc, in_maps, core_ids=list(range(N_CORES)), trace=trace
    )
    shards = []
    for k in range(N_CORES):
        oh = res.results[k]["outh"]  # [NST, P, SPT, D] bf16, row = (s*SPT+t)*128+p
        flat = (
            oh.astype(np.float32).transpose(0, 2, 1, 3).reshape(M, D)
        )
        shards.append(flat.reshape(B_PER_CORE, L, D))
    out = np.concatenate(shards, axis=0)
    if trace:
        kernel.last_result = res
    return out


# revision 31
# speedup vs baseline: 1.2028x; 1.0540x over previous
"""Trainium2 Bass kernel for nn_AddPoolingFusion.

Reference computation (b=16, l1=l2=2048, d1=d2=d3=768):
    y1  = x1 @ W1.T + b1                      # [b, l1, d3]
    y2  = x2 @ W2.T + b2                      # [b, l2, d3]
    out = y1 + mean(y2, axis=1, keepdims=True)

Because the mean over l2 commutes with the linear layer:
    out[b,i,:] = x1[b,i] @ W1.T + c[b]
    c[b]       = (b1 + b2) + mean_j(x2[b,j]) @ W2.T

Strategy: data-parallel over batch, 2 batches per core, no collectives.
Per core the only heavy compute is the x1 matmul (bf16 on TensorE) and
the x2 mean (DVE accumulate + tiny ones-matmul partition reduce); the
kernel is HBM-bandwidth / TensorE bound.

Host-side prep (layout/dtype only): all tensors are pre-arranged into
partition-major SBUF images so every DMA descriptor covers a 6-48 KB
contiguous run (small runs make the SDMA engines descriptor-bound at
~240 GB/s; big runs reach HBM line rate). x1 is pre-transposed so the
contraction dim lands on SBUF partitions, and x1/x2/weights are pre-cast
to bf16 (the TensorE matmul runs in bf16 either way; rel err stays ~3e-3
vs the 2e-2 gate). The output is stored as bf16 and upcast on the host.
"""

import os
import sys

import numpy as np

# concourse normally comes from the axon site overlay already on sys.path;
# append /opt/trn_rl_repo as a fallback only.
if "/opt/trn_rl_repo" not in sys.path:
    sys.path.append("/opt/trn_rl_repo")

N_CORES = 8
B_PER_CORE = 2
L = 2048
D = 768  # d1 == d2 == d3 == 768
P = 128
NCH = D // P  # 6 contraction chunks
M = B_PER_CORE * L  # 4096 rows per core
TPG = 8  # m-tiles per x1 DMA group
NGRP = (M // P) // TPG  # 4 groups
SPT = 4  # m-tiles per output store
NST = (M // P) // SPT  # 8 stores


def build_nc(debug=False, explicit_ldw=False):
    import concourse.bacc as bacc
    import concourse.mybir as mybir
    import concourse.tile as tile

    f32 = mybir.dt.float32
    bf16 = mybir.dt.bfloat16
    add = mybir.AluOpType.add

    nc = bacc.Bacc(None, target_bir_lowering=False, debug=debug)

    # All layouts are partition-major SBUF images (see make_in_maps).
    x1h = nc.declare_dram_parameter("x1h", [NGRP, P, NCH, TPG * P], bf16, isOutput=False)
    x2h = nc.declare_dram_parameter("x2h", [B_PER_CORE, P, L // P, D], bf16, isOutput=False)
    w1h = nc.declare_dram_parameter("w1h", [P, NCH, D], bf16, isOutput=False)
    w2h = nc.declare_dram_parameter("w2h", [P, NCH, D], bf16, isOutput=False)
    bsum = nc.declare_dram_parameter("bsum", [1, D], f32, isOutput=False)
    outh = nc.declare_dram_parameter("outh", [NST, P, SPT, D], bf16, isOutput=True)

    with tile.TileContext(nc) as tc:
        with (
            tc.tile_pool(name="const", bufs=1) as const,
            tc.tile_pool(name="x2p", bufs=4) as x2p,
            tc.tile_pool(name="x2big", bufs=1) as x2big,
            tc.tile_pool(name="x1p", bufs=4) as x1p,
            tc.tile_pool(name="yp", bufs=2) as yp,
            tc.tile_pool(name="small", bufs=1) as small,
            tc.tile_pool(name="psY", bufs=3, space="PSUM") as psY,
            tc.tile_pool(name="psM", bufs=1, space="PSUM") as psM,
        ):
            # ---- weights / constants ----
            # DMA ring split: x1 + W1 on the Sync HWDGE ring; x2 + W2 +
            # stores on the Scalar HWDGE ring.
            w1sb = const.tile([P, NCH, D], bf16)
            nc.scalar.dma_start(w1sb[:], w1h[:])
            ones_sb = const.tile([P, 1], bf16)
            nc.vector.memset(ones_sb[:], 1.0)

            # ---- x2 loads + per-batch accumulate (DVE) ----
            # Ring FIFO order on the Scalar ring encodes the schedule:
            # x2 batch 0 (c[0]-critical) -> w2/bsum -> x2 batch 1 -> stores.
            accs = []

            def x2_acc(b, nchunk, pool):
                with nc.named_scope(f"x2_acc{b}"):
                    acc = small.tile([P, D], bf16, tag=f"acc{b}")
                    tpc = (L // P) // nchunk
                    first = True
                    for h in range(nchunk):
                        st = pool.tile([P, tpc, D], bf16, tag=f"x2st{b}")
                        dma = nc.scalar.dma_start(
                            st[:], x2h[b, :, tpc * h : tpc * (h + 1), :]
                        )
                        for t in range(tpc):
                            if first:
                                nc.vector.tensor_copy(acc[:], st[:, 0, :])
                                first = False
                            else:
                                nc.vector.tensor_tensor(
                                    acc[:], acc[:], st[:, t, :], op=add
                                )
                    accs.append(acc)
                    return dma

            x2_acc(0, 4, x2p)
            w2sb = const.tile([P, NCH, D], bf16)
            nc.scalar.dma_start(w2sb[:], w2h[:])
            bsum_sb = const.tile([1, D], f32)
            nc.scalar.dma_start(bsum_sb[:], bsum[:])
            b1_dma = x2_acc(1, 1, x2big)

            def c_path(b):
                # partition-major sum: xbt[p, c] = (1/L) sum_j x2[b, j, c*128+p]
                with nc.named_scope(f"c_path{b}"):
                    acc = accs[b]
                    xbt = small.tile([P, NCH], bf16, tag=f"xbt{b}")
                    for c in range(NCH):
                        px = psM.tile([P, 1], f32, tag="pscratch")
                        nc.tensor.matmul(
                            px[:], acc[:, c * P : (c + 1) * P], ones_sb[:],
                            start=True, stop=True,
                        )
                        # fold the 1/L mean scale in via the copy
                        nc.vector.tensor_scalar_mul(xbt[:, c : c + 1], px[:], 1.0 / L)
                    # c_lin = xbar2 @ W2.T  (tiny matmul, K=768, M=1, N=768)
                    pc = psM.tile([1, D], f32, tag="pscratch")
                    for c in range(NCH):
                        nc.tensor.matmul(
                            pc[:, 0:512], xbt[:, c : c + 1], w2sb[:, c, 0:512],
                            start=(c == 0), stop=(c == NCH - 1),
                        )
                    for c in range(NCH):
                        nc.tensor.matmul(
                            pc[:, 512:768], xbt[:, c : c + 1], w2sb[:, c, 512:768],
                            start=(c == 0), stop=(c == NCH - 1),
                        )
                    cs = small.tile([1, D], bf16, tag=f"cs{b}")
                    nc.vector.tensor_tensor(cs[:], pc[:], bsum_sb[:], op=add)
                    cr = small.tile([P, D], bf16, tag=f"cr{b}")
                    nc.gpsimd.partition_broadcast(cr[:], cs[:])
                    return cr

            # ---- main matmul: out = x1 @ W1.T + c[b] ----
            c_rep = [None, None]

            def group(g, defer_bias=False, prev_dma=None):
                with nc.named_scope(f"grp{g}"):
                    xs = x1p.tile([P, NCH, TPG * P], bf16, tag="xs")
                    dma = nc.sync.dma_start(xs[:], x1h[g])
                    if prev_dma is not None:
                        # just-in-time pacing: an x1 load starts only after an
                        # earlier one fully lands, so queued loads never steal
                        # HBM bandwidth from the transfer TensorE waits on
                        tile.add_dep_helper(
                            dma.ins, prev_dma.ins, sync=True,
                            reason="pace x1 loads just-in-time",
                        )
                    ys = yp.tile([P, TPG, D], bf16)
                    for t in range(TPG):
                        mt = g * TPG + t
                        b = mt // (L // P)
                        py_ = psY.tile([P, D], f32)
                        xw = xs[:, :, t * P : (t + 1) * P]
                        for c in range(NCH):
                            if explicit_ldw:
                                nc.tensor.ldweights(xw[:, c, :])
                            nc.tensor.matmul(
                                py_[:, 0:512], xw[:, c, :], w1sb[:, c, 0:512],
                                start=(c == 0), stop=(c == NCH - 1),
                            )
                            nc.tensor.matmul(
                                py_[:, 512:768], xw[:, c, :], w1sb[:, c, 512:768],
                                start=(c == 0), stop=(c == NCH - 1),
                            )
                        if defer_bias:
                            # plain evac: never lets PSUM recycling wait on c
                            nc.vector.tensor_copy(ys[:, t, :], py_[:])
                        else:
                            # fused evac + bias add
                            nc.vector.tensor_tensor(
                                ys[:, t, :], py_[:], c_rep[b][:], op=add
                            )
                            if t % SPT == SPT - 1:
                                nc.sync.dma_start(
                                    outh[mt // SPT], ys[:, t - SPT + 1 : t + 1, :]
                                )
                    group.last_dma = dma
                    if defer_bias:
                        b = (g * TPG) // (L // P)
                        for t in range(TPG):
                            mt = g * TPG + t
                            nc.vector.tensor_tensor(
                                ys[:, t, :], ys[:, t, :], c_rep[b][:], op=add
                            )
                            if t % SPT == SPT - 1:
                                nc.sync.dma_start(
                                    outh[mt // SPT], ys[:, t - SPT + 1 : t + 1, :]
                                )

            # batch-0 c-path first (gates groups 0-1), batch-1 c-path after
            # group 1 so its PE slot lands when its inputs are long ready.
            # Group 0 runs while c[0] is still in flight -> deferred bias.
            c_rep[0] = c_path(0)
            group(0, defer_bias=True)
            group(1, prev_dma=group.last_dma)
            c_rep[1] = c_path(1)
            group(2, defer_bias=True, prev_dma=group.last_dma)
            group(3, prev_dma=group.last_dma)
            # x2 batch 1 is the least latency-critical load: let every x1
            # group go first (c[1] is still ready before group 3's evacs)
            tile.add_dep_helper(
                b1_dma.ins, group.last_dma.ins, sync=True,
                reason="x2 b1 yields HBM BW to x1 stream",
            )

    return nc


def make_in_maps(x1, x2, W1, b1, W2, b2):
    import ml_dtypes

    bf16 = ml_dtypes.bfloat16

    def wlayout(W):
        # [e, d] -> W.T [d, e] -> [p, c, e] with d = c*128 + p
        wt = np.ascontiguousarray(W.T).reshape(NCH, P, D).transpose(1, 0, 2)
        return np.ascontiguousarray(wt).astype(bf16)

    w1h = wlayout(W1)
    w2h = wlayout(W2)
    bsum_h = np.ascontiguousarray((b1 + b2).reshape(1, D).astype(np.float32))
    in_maps = []
    for k in range(N_CORES):
        x1_s = x1[k * B_PER_CORE : (k + 1) * B_PER_CORE]  # [2, 2048, 768]
        x2_s = x2[k * B_PER_CORE : (k + 1) * B_PER_CORE]
        # x1t [d, m] with col m = b*2048 + i, then group-major partition image
        x1t = np.transpose(x1_s, (2, 0, 1)).reshape(D, M)
        x1h = np.ascontiguousarray(
            x1t.reshape(NCH, P, NGRP, TPG * P).transpose(2, 1, 0, 3)
        ).astype(bf16)  # [g, p, c, m_in_group]
        # x2 [b, j, d] with j = t*128 + p -> [b, p, t, d]
        x2h = np.ascontiguousarray(
            x2_s.reshape(B_PER_CORE, L // P, P, D).transpose(0, 2, 1, 3)
        ).astype(bf16)
        in_maps.append(
            {"x1h": x1h, "x2h": x2h, "w1h": w1h, "w2h": w2h, "bsum": bsum_h}
        )
    return in_maps


def kernel(x1, x2, W1, b1, W2, b2, trace=False, explicit_ldw=False):
    from concourse.bass_utils import run_bass_kernel_spmd

    nc = build_nc(debug=False, explicit_ldw=explicit_ldw)
    nc.finalize()
    in_maps = make_in_maps(x1, x2, W1, b1, W2, b2)
    res = run_bass_kernel_spmd(
        nc, in_maps, core_ids=list(range(N_CORES)), trace=trace
    )
    shards = []
    for k in range(N_CORES):
        oh = res.results[k]["outh"]  # [NST, P, SPT, D] bf16, row = (s*SPT+t)*128+p
        flat = (
            oh.astype(np.float32).transpose(0, 2, 1, 3).reshape(M, D)
        )
        shards.append(flat.reshape(B_PER_CORE, L, D))
    out = np.concatenate(shards, axis=0)
    if trace:
        kernel.last_result = res
    return out
